# revision 3
# baseline (speedup 1.0000x reference)
"""Trainium2 Bass kernel for nn_BlockSparseMoE (top-2 of 8 experts, SwiGLU).

Strategy (expert-parallel, sparse dispatch):
  - Host: compute router (gate matmul + softmax + top-2 + renorm) in fp64,
    gather each expert's tokens into a capacity-padded batch (the
    "all-to-all dispatch by selected expert" happens at input-sharding
    time, which is host-side by construction).
  - Device (SPMD x8, one expert per core): xT [D, C] bf16 streams through
    w1/w3 (SwiGLU) and w2 in bf16 with fp32 PSUM accumulation, rows scaled
    by the renormalized top-2 weight. No collectives needed.
  - Host: scatter-add the two expert contributions per token.

Layout per core:
  phase A: hT[f, t] = silu(x@w1)^T * (x@w3)^T computed directly transposed
           (lhsT = w1 d-chunk [128, 128f], rhs = xT d-chunk [128, tchunk])
           so no on-device transposes are ever needed.
  phase B: y[t, d] accumulated over 32 f-chunks (lhsT = hT f-chunk, rhs =
           w2 f-chunk), scaled by combine weight via per-partition scalar.
"""

import numpy as np
import ml_dtypes

HIDDEN = 1024
FFN = 4096
NUM_EXPERTS = 8
TOP_K = 2
N_CORES = 8

_BF16 = ml_dtypes.bfloat16
_nc_cache = {}


# ---------------------------------------------------------------- router ----
def _route(x, gate_w, gate_b):
    """Top-2 routing. Returns per-expert (token_idx, renorm_weight)."""
    logits = x.astype(np.float64) @ gate_w.astype(np.float64) + gate_b.astype(
        np.float64
    )
    logits -= logits.max(axis=-1, keepdims=True)
    p = np.exp(logits)
    p /= p.sum(axis=-1, keepdims=True)
    # top-2 by prob, ties broken by lower index (matches jax.lax.top_k)
    top2 = np.argsort(-p, axis=-1, kind="stable")[:, :TOP_K]
    pt = np.take_along_axis(p, top2, axis=-1)
    wt = pt / pt.sum(axis=-1, keepdims=True)
    idxs, wts = [], []
    for e in range(NUM_EXPERTS):
        mask = top2 == e  # [T, 2]
        tok = np.nonzero(mask.any(axis=-1))[0]
        w = wt[tok, np.argmax(mask[tok], axis=-1)]
        idxs.append(tok)
        wts.append(w.astype(np.float32))
    return idxs, wts


# ------------------------------------------------------------- device IR ----
def _build(C, chunk):
    """Build the per-core Bacc graph for capacity C (= chunk * n_chunks)."""
    import concourse.bacc as bacc
    import concourse.bass as bass
    import concourse.mybir as mybir
    import concourse.tile as tile

    n_chunks = C // chunk
    n_subs = chunk // 128  # 128-token sub-tiles per chunk
    DC = HIDDEN // 128  # 8 contraction chunks for x@w1
    FT = FFN // 128  # 32 f-tiles
    FG = FFN // 512  # 8 f-groups (512 wide)
    DO = HIDDEN // 512  # 2 output-d chunks

    bf16 = mybir.dt.bfloat16
    f32 = mybir.dt.float32

    nc = bacc.Bacc("TRN2", target_bir_lowering=False, debug=False,
                   num_devices=N_CORES)

    xT_d = nc.dram_tensor("xT", [HIDDEN, C], bf16, kind="ExternalInput")
    w1_d = nc.dram_tensor("w1", [HIDDEN, FFN], bf16, kind="ExternalInput")
    w3_d = nc.dram_tensor("w3", [HIDDEN, FFN], bf16, kind="ExternalInput")
    w2_d = nc.dram_tensor("w2", [FFN, HIDDEN], bf16, kind="ExternalInput")
    s_d = nc.dram_tensor("s", [C], f32, kind="ExternalInput")
    y_d = nc.dram_tensor("y", [C, HIDDEN], f32, kind="ExternalOutput")

    # DRAM views tiled for 128-partition DMA
    xT_v = xT_d.ap().rearrange("(dc p) c -> p dc c", p=128)
    w1_v = w1_d.ap().rearrange("(dc p) f -> p dc f", p=128)
    w3_v = w3_d.ap().rearrange("(dc p) f -> p dc f", p=128)
    w2_v = w2_d.ap().rearrange("(ft p) d -> p ft d", p=128)
    s_v = s_d.ap().rearrange("(j p) -> p j", p=128)

    with tile.TileContext(nc) as tc:
        with (
            tc.tile_pool(name="res", bufs=1) as res,
            tc.tile_pool(name="w13", bufs=2) as w13,
            tc.tile_pool(name="hp", bufs=2) as hp,
            tc.tile_pool(name="sil", bufs=4) as silp,
            tc.tile_pool(name="yo", bufs=4) as yop,
            tc.tile_pool(name="ps", bufs=2, space=bass.MemorySpace.PSUM) as ps,
            tc.tile_pool(name="yps", bufs=4, space=bass.MemorySpace.PSUM) as yps,
        ):
            # resident tensors
            xT = res.tile([128, DC, C], bf16, tag="xT")
            w2 = res.tile([128, FT, HIDDEN], bf16, tag="w2")
            s_sb = res.tile([128, C // 128], f32, tag="s")
            nc.sync.dma_start(xT[:], xT_v)
            nc.sync.dma_start(s_sb[:], s_v)
            # split the 8MB w2 load into 4 DMAs so it pipelines
            for q in range(4):
                nc.sync.dma_start(w2[:, q * 8:(q + 1) * 8, :],
                                  w2_v[:, q * 8:(q + 1) * 8, :])

            for t in range(n_chunks):
                t0 = t * chunk
                hT = hp.tile([128, FT, chunk], bf16, tag="hT")
                # ---- phase A: hT[f, t] for this token chunk ----
                for fg in range(FG):
                    w1_sb = w13.tile([128, DC, 512], bf16, tag="w1")
                    w3_sb = w13.tile([128, DC, 512], bf16, tag="w3")
                    nc.sync.dma_start(w1_sb[:], w1_v[:, :, fg * 512:(fg + 1) * 512])
                    nc.sync.dma_start(w3_sb[:], w3_v[:, :, fg * 512:(fg + 1) * 512])
                    for fl in range(4):
                        ft = fg * 4 + fl
                        ph1 = ps.tile([128, chunk], f32, tag="ph1")
                        ph3 = ps.tile([128, chunk], f32, tag="ph3")
                        for dc in range(DC):
                            nc.tensor.matmul(
                                ph1[:],
                                w1_sb[:, dc, fl * 128:(fl + 1) * 128],
                                xT[:, dc, t0:t0 + chunk],
                                start=(dc == 0), stop=(dc == DC - 1),
                            )
                        for dc in range(DC):
                            nc.tensor.matmul(
                                ph3[:],
                                w3_sb[:, dc, fl * 128:(fl + 1) * 128],
                                xT[:, dc, t0:t0 + chunk],
                                start=(dc == 0), stop=(dc == DC - 1),
                            )
                        sil = silp.tile([128, chunk], bf16, tag="sil")
                        nc.scalar.activation(
                            sil[:], ph1[:], mybir.ActivationFunctionType.Silu
                        )
                        nc.vector.tensor_mul(hT[:, ft, :], sil[:], ph3[:])

                # ---- phase B: y[t, d] for this chunk ----
                for tsub in range(n_subs):
                    j = t * n_subs + tsub
                    ypsl = [
                        yps.tile([128, 512], f32, tag="yp", name=f"yp{j}_{do}")
                        for do in range(DO)
                    ]
                    for f in range(FT):
                        for do in range(DO):
                            nc.tensor.matmul(
                                ypsl[do][:],
                                hT[:, f, tsub * 128:(tsub + 1) * 128],
                                w2[:, f, do * 512:(do + 1) * 512],
                                start=(f == 0), stop=(f == FT - 1),
                            )
                    for do in range(DO):
                        ysb = yop.tile([128, 512], f32, tag="ysb")
                        nc.vector.tensor_scalar_mul(
                            ysb[:], ypsl[do][:], s_sb[:, j:j + 1]
                        )
                        nc.sync.dma_start(
                            y_d[j * 128:(j + 1) * 128, do * 512:(do + 1) * 512],
                            ysb[:],
                        )
    nc.compile()
    return nc


def _get_nc(C, chunk):
    key = (C, chunk)
    if key not in _nc_cache:
        _nc_cache[key] = _build(C, chunk)
    return _nc_cache[key]


def _capacity(max_load):
    """Pick capacity C (multiple of 128) and chunk (<=512, multiple of 128)."""
    n = max(1, -(-max_load // 512))  # ceil
    chunk = -(-max_load // (n * 128)) * 128
    return chunk * n, chunk


# ---------------------------------------------------------------- kernel ----
def kernel(hidden_states, gate_w, gate_b, w1, w3, w2, _trace=False):
    from concourse.bass_utils import run_bass_kernel_spmd

    B, S, D = hidden_states.shape
    T = B * S
    x = np.asarray(hidden_states, np.float32).reshape(T, D)
    idxs, wts = _route(x, np.asarray(gate_w, np.float32),
                       np.asarray(gate_b, np.float32))
    C, chunk = _capacity(max(len(i) for i in idxs))
    nc = _get_nc(C, chunk)

    w1 = np.asarray(w1)
    w3 = np.asarray(w3)
    w2 = np.asarray(w2)
    in_maps = []
    for e in range(NUM_EXPERTS):
        tok, wt = idxs[e], wts[e]
        l = len(tok)
        xT = np.zeros((D, C), _BF16)
        xT[:, :l] = x[tok].T.astype(_BF16)
        s = np.zeros((C,), np.float32)
        s[:l] = wt
        in_maps.append({
            "xT": xT,
            "w1": np.ascontiguousarray(w1[e]).astype(_BF16),
            "w3": np.ascontiguousarray(w3[e]).astype(_BF16),
            "w2": np.ascontiguousarray(w2[e]).astype(_BF16),
            "s": s,
        })

    res = run_bass_kernel_spmd(nc, in_maps, core_ids=list(range(N_CORES)),
                               trace=_trace)

    out = np.zeros((T, D), np.float32)
    for e in range(NUM_EXPERTS):
        tok = idxs[e]
        out[tok] += res.results[e]["y"][: len(tok)]
    out = out.reshape(B, S, D)
    if _trace:
        return out, res
    return out


# revision 4
# speedup vs baseline: 1.0699x; 1.0699x over previous
"""Trainium2 Bass kernel for nn_BlockSparseMoE (top-2 of 8 experts, SwiGLU).

Strategy (expert-parallel, sparse dispatch):
  - Host: compute router (gate matmul + softmax + top-2 + renorm) in fp64,
    gather each expert's tokens into a capacity-padded batch (the
    "all-to-all dispatch by selected expert" happens at input-sharding
    time, which is host-side by construction).
  - Device (SPMD x8, one expert per core): xT [D, C] bf16 streams through
    w1/w3 (SwiGLU) and w2 in bf16 with fp32 PSUM accumulation, rows scaled
    by the renormalized top-2 weight. No collectives needed.
  - Host: scatter-add the two expert contributions per token.

Layout per core:
  phase A: hT[f, t] = silu(x@w1)^T * (x@w3)^T computed directly transposed
           (lhsT = w1 d-chunk [128, 128f], rhs = xT d-chunk [128, tchunk])
           so no on-device transposes are ever needed.
  phase B: y[t, d] accumulated over 32 f-chunks (lhsT = hT f-chunk, rhs =
           w2 f-chunk), scaled by combine weight via per-partition scalar.
"""

import numpy as np
import ml_dtypes

HIDDEN = 1024
FFN = 4096
NUM_EXPERTS = 8
TOP_K = 2
N_CORES = 8

_BF16 = ml_dtypes.bfloat16
_nc_cache = {}


# ---------------------------------------------------------------- router ----
def _route(x, gate_w, gate_b):
    """Top-2 routing. Returns per-expert (token_idx, renorm_weight)."""
    logits = x.astype(np.float64) @ gate_w.astype(np.float64) + gate_b.astype(
        np.float64
    )
    logits -= logits.max(axis=-1, keepdims=True)
    p = np.exp(logits)
    p /= p.sum(axis=-1, keepdims=True)
    # top-2 by prob, ties broken by lower index (matches jax.lax.top_k)
    top2 = np.argsort(-p, axis=-1, kind="stable")[:, :TOP_K]
    pt = np.take_along_axis(p, top2, axis=-1)
    wt = pt / pt.sum(axis=-1, keepdims=True)
    idxs, wts = [], []
    for e in range(NUM_EXPERTS):
        mask = top2 == e  # [T, 2]
        tok = np.nonzero(mask.any(axis=-1))[0]
        w = wt[tok, np.argmax(mask[tok], axis=-1)]
        idxs.append(tok)
        wts.append(w.astype(np.float32))
    return idxs, wts


# ------------------------------------------------------------- device IR ----
def _build(C, chunk):
    """Build the per-core Bacc graph for capacity C (= chunk * n_chunks)."""
    import concourse.bacc as bacc
    import concourse.bass as bass
    import concourse.mybir as mybir
    import concourse.tile as tile

    n_chunks = C // chunk
    n_subs = chunk // 128  # 128-token sub-tiles per chunk
    DC = HIDDEN // 128  # 8 contraction chunks for x@w1
    FT = FFN // 128  # 32 f-tiles
    FG = FFN // 512  # 8 f-groups (512 wide)
    DO = HIDDEN // 512  # 2 output-d chunks

    bf16 = mybir.dt.bfloat16
    f32 = mybir.dt.float32

    nc = bacc.Bacc("TRN2", target_bir_lowering=False, debug=False,
                   num_devices=N_CORES)

    xT_d = nc.dram_tensor("xT", [HIDDEN, C], bf16, kind="ExternalInput")
    w1_d = nc.dram_tensor("w1", [HIDDEN, FFN], bf16, kind="ExternalInput")
    w3_d = nc.dram_tensor("w3", [HIDDEN, FFN], bf16, kind="ExternalInput")
    w2_d = nc.dram_tensor("w2", [FFN, HIDDEN], bf16, kind="ExternalInput")
    s_d = nc.dram_tensor("s", [C], f32, kind="ExternalInput")
    y_d = nc.dram_tensor("y", [C, HIDDEN], f32, kind="ExternalOutput")

    # DRAM views tiled for 128-partition DMA
    xT_v = xT_d.ap().rearrange("(dc p) c -> p dc c", p=128)
    w1_v = w1_d.ap().rearrange("(dc p) f -> p dc f", p=128)
    w3_v = w3_d.ap().rearrange("(dc p) f -> p dc f", p=128)
    w2_v = w2_d.ap().rearrange("(ft p) d -> p ft d", p=128)
    s_v = s_d.ap().rearrange("(j p) -> p j", p=128)

    with tile.TileContext(nc) as tc:
        with (
            tc.tile_pool(name="res", bufs=1) as res,
            tc.tile_pool(name="w13", bufs=2) as w13,
            tc.tile_pool(name="hp", bufs=2) as hp,
            tc.tile_pool(name="sil", bufs=4) as silp,
            tc.tile_pool(name="yo", bufs=4) as yop,
            tc.tile_pool(name="ps", bufs=2, space=bass.MemorySpace.PSUM) as ps,
            tc.tile_pool(name="yps", bufs=4, space=bass.MemorySpace.PSUM) as yps,
        ):
            # resident tensors.  DMA order matters: the PE's first work is
            # chunk-0 phase A, which needs xT[:, :, :chunk] and w1/w3 of
            # fg 0 — so those transfers go first; the 8MB w2 (needed only
            # for phase B, ~85us in) is interleaved across chunk-0's
            # f-group loop so it never blocks the critical path.
            xT = res.tile([128, DC, C], bf16, tag="xT")
            w2 = res.tile([128, FT, HIDDEN], bf16, tag="w2")
            s_sb = res.tile([128, C // 128], f32, tag="s")
            nc.sync.dma_start(xT[:, :, 0:chunk], xT_v[:, :, 0:chunk])

            for t in range(n_chunks):
                t0 = t * chunk
                hT = hp.tile([128, FT, chunk], bf16, tag="hT")
                # ---- phase A: hT[f, t] for this token chunk ----
                for fg in range(FG):
                    w1_sb = w13.tile([128, DC, 512], bf16, tag="w1")
                    w3_sb = w13.tile([128, DC, 512], bf16, tag="w3")
                    nc.sync.dma_start(w1_sb[:], w1_v[:, :, fg * 512:(fg + 1) * 512])
                    nc.sync.dma_start(w3_sb[:], w3_v[:, :, fg * 512:(fg + 1) * 512])
                    if t == 0:
                        # stream w2 (4 ft-rows = 1MB per fg) behind the
                        # critical w1/w3 loads; complete before phase B
                        nc.sync.dma_start(w2[:, fg * 4:(fg + 1) * 4, :],
                                          w2_v[:, fg * 4:(fg + 1) * 4, :])
                        if fg == 0:
                            nc.sync.dma_start(s_sb[:], s_v)
                        if fg + 1 < n_chunks:
                            # prefetch next chunk's xT slice
                            tn = fg + 1
                            nc.sync.dma_start(
                                xT[:, :, tn * chunk:(tn + 1) * chunk],
                                xT_v[:, :, tn * chunk:(tn + 1) * chunk],
                            )
                    for fl in range(4):
                        ft = fg * 4 + fl
                        ph1 = ps.tile([128, chunk], f32, tag="ph1")
                        ph3 = ps.tile([128, chunk], f32, tag="ph3")
                        for dc in range(DC):
                            nc.tensor.matmul(
                                ph1[:],
                                w1_sb[:, dc, fl * 128:(fl + 1) * 128],
                                xT[:, dc, t0:t0 + chunk],
                                start=(dc == 0), stop=(dc == DC - 1),
                            )
                        for dc in range(DC):
                            nc.tensor.matmul(
                                ph3[:],
                                w3_sb[:, dc, fl * 128:(fl + 1) * 128],
                                xT[:, dc, t0:t0 + chunk],
                                start=(dc == 0), stop=(dc == DC - 1),
                            )
                        sil = silp.tile([128, chunk], bf16, tag="sil")
                        nc.scalar.activation(
                            sil[:], ph1[:], mybir.ActivationFunctionType.Silu
                        )
                        nc.vector.tensor_mul(hT[:, ft, :], sil[:], ph3[:])

                # ---- phase B: y[t, d] for this chunk ----
                for tsub in range(n_subs):
                    j = t * n_subs + tsub
                    ypsl = [
                        yps.tile([128, 512], f32, tag="yp", name=f"yp{j}_{do}")
                        for do in range(DO)
                    ]
                    for f in range(FT):
                        for do in range(DO):
                            nc.tensor.matmul(
                                ypsl[do][:],
                                hT[:, f, tsub * 128:(tsub + 1) * 128],
                                w2[:, f, do * 512:(do + 1) * 512],
                                start=(f == 0), stop=(f == FT - 1),
                            )
                    for do in range(DO):
                        ysb = yop.tile([128, 512], f32, tag="ysb")
                        nc.vector.tensor_scalar_mul(
                            ysb[:], ypsl[do][:], s_sb[:, j:j + 1]
                        )
                        nc.sync.dma_start(
                            y_d[j * 128:(j + 1) * 128, do * 512:(do + 1) * 512],
                            ysb[:],
                        )
    nc.compile()
    return nc


def _get_nc(C, chunk):
    key = (C, chunk)
    if key not in _nc_cache:
        _nc_cache[key] = _build(C, chunk)
    return _nc_cache[key]


def _capacity(max_load):
    """Pick capacity C (multiple of 128) and chunk (<=512, multiple of 128)."""
    n = max(1, -(-max_load // 512))  # ceil
    chunk = -(-max_load // (n * 128)) * 128
    return chunk * n, chunk


# ---------------------------------------------------------------- kernel ----
def kernel(hidden_states, gate_w, gate_b, w1, w3, w2, _trace=False):
    from concourse.bass_utils import run_bass_kernel_spmd

    B, S, D = hidden_states.shape
    T = B * S
    x = np.asarray(hidden_states, np.float32).reshape(T, D)
    idxs, wts = _route(x, np.asarray(gate_w, np.float32),
                       np.asarray(gate_b, np.float32))
    C, chunk = _capacity(max(len(i) for i in idxs))
    nc = _get_nc(C, chunk)

    w1 = np.asarray(w1)
    w3 = np.asarray(w3)
    w2 = np.asarray(w2)
    in_maps = []
    for e in range(NUM_EXPERTS):
        tok, wt = idxs[e], wts[e]
        l = len(tok)
        xT = np.zeros((D, C), _BF16)
        xT[:, :l] = x[tok].T.astype(_BF16)
        s = np.zeros((C,), np.float32)
        s[:l] = wt
        in_maps.append({
            "xT": xT,
            "w1": np.ascontiguousarray(w1[e]).astype(_BF16),
            "w3": np.ascontiguousarray(w3[e]).astype(_BF16),
            "w2": np.ascontiguousarray(w2[e]).astype(_BF16),
            "s": s,
        })

    res = run_bass_kernel_spmd(nc, in_maps, core_ids=list(range(N_CORES)),
                               trace=_trace)

    out = np.zeros((T, D), np.float32)
    for e in range(NUM_EXPERTS):
        tok = idxs[e]
        out[tok] += res.results[e]["y"][: len(tok)]
    out = out.reshape(B, S, D)
    if _trace:
        return out, res
    return out
